# revision 1
# baseline (speedup 1.0000x reference)
"""Trainium2 Bass kernel for CrossAttention (B=8, N=M=2048, C=1024), fp32.

Sharding: data-parallel — one batch element per NeuronCore (8 cores).

Per-core computation (batch b):
  q  = x[b] @ wq^T          -> held transposed:  qT[d, n]
  kT[d, m] = (support[b] @ wk^T)^T
  v[m, d]  = (support[b] @ wv^T) * mask[m]   (post-softmax column mask == row
                                              mask on v; softmax denominator
                                              still spans all m)
  p  = exp(SCALE * qT^T kT)  (no max subtraction: logits ~ N(0, 8), safe fp32)
  o[n, d] = (p @ v) / rowsum(p)
  The reference's  out.swapaxes(1,2).reshape(N, C)  permutation satisfies
  o_perm[2t+i, c] = o[1024*i + c, t], so the final projection becomes
  final[2t+i, d'] = sum_c o[1024*i + c, t] * proj_w[d', c]  — a plain matmul
  with o-block-i rows as the contraction dim, written out with row stride 2.

Matmul operands are float32r (full-rate PE streaming; plain fp32 is 4 cyc/row).
Host-side prep transposes x/support/weights once (fp32 has no DMA-transpose on
TRN2) and lays weights out in consumption order so chunked DMAs pipeline with
the first accumulation groups at phase boundaries.
"""

import sys

sys.path.insert(0, "/opt/trn_rl_repo")

import numpy as np

import concourse.bass as bass
import concourse.tile as tile
from concourse import bacc, mybir
from concourse.bass_utils import run_bass_kernel_spmd
from concourse.masks import make_identity

F32 = mybir.dt.float32
F32R = mybir.dt.float32r
AF = mybir.ActivationFunctionType

B, N, M, C = 8, 2048, 2048, 1024
CT = C // 128          # 8 c-tiles (contraction / channel partition tiles)
MT = M // 128          # 16 m-tiles
SCALE = (C // 8) ** -0.5
NCHUNK = 256           # q rows computed per chunk in the attention phase
MS = 512               # m-chunk for kv build and the s matmul free dim
WCH = 16               # weight DMA chunks (arrival order == consumption order)

_CACHE = {}


def _build_program():
    nc = bacc.Bacc(
        "TRN2",
        target_bir_lowering=False,
        debug=False,
        enable_asserts=False,
        num_devices=8,
    )

    xT = nc.dram_tensor("xT", [128, CT, N], F32, kind="ExternalInput")
    sT = nc.dram_tensor("sT", [128, CT, M], F32, kind="ExternalInput")
    wqT = nc.dram_tensor("wqT", [128, CT * C], F32, kind="ExternalInput")
    wkT = nc.dram_tensor("wkT", [128, CT * C], F32, kind="ExternalInput")
    wvT = nc.dram_tensor("wvT", [128, CT * C], F32, kind="ExternalInput")
    pwT = nc.dram_tensor("pwT", [128, CT * C], F32, kind="ExternalInput")
    maskf = nc.dram_tensor("maskf", [128, MT], F32, kind="ExternalInput")
    biasb = nc.dram_tensor("biasb", [128, C], F32, kind="ExternalInput")
    out = nc.dram_tensor("out", [N, C], F32, kind="ExternalOutput")

    with tile.TileContext(nc, pool_alloc_mode="queue") as tc:
        _trace_kernel(tc, xT, sT, wqT, wkT, wvT, pwT, maskf, biasb, out)
    nc.compile()
    return nc


def _dma_w(nc, wtile, wdram, ch0=0, ch1=WCH):
    # chunked weight load: pipelines with the first consuming matmul groups
    cw = (CT * C) // WCH
    for ch in range(ch0, ch1):
        nc.sync.dma_start(
            wtile[:, ch * cw:(ch + 1) * cw],
            wdram[:, ch * cw:(ch + 1) * cw].bitcast(F32R),
        )


def _dma_act(nc, atile, adram, col0, cols):
    # per-ct chunked activation load (arrival order == psum-group order)
    for ct in range(CT):
        nc.sync.dma_start(
            atile[:, ct, :],
            adram[:, ct, col0:col0 + cols].bitcast(F32R),
        )


def _trace_kernel(tc, xT, sT, wqT, wkT, wvT, pwT, maskf, biasb, out):
    nc = tc.nc

    from contextlib import ExitStack

    with ExitStack() as ctx:
        persist = ctx.enter_context(tc.tile_pool(name="persist", bufs=1))

        ident0 = persist.tile([128, 128], F32, tag="ident0")
        make_identity(nc, ident0[:])
        ident = persist.tile([128, 128], F32R, tag="ident")
        nc.scalar.copy(ident[:], ident0[:])
        maskt = persist.tile([128, MT], F32, tag="maskt")
        nc.sync.dma_start(maskt[:], maskf[:])

        # o bounce buffer in DRAM (dependency-tracked tile)
        dram = ctx.enter_context(tc.tile_pool(name="dram", bufs=1, space="DRAM"))
        o_dram = dram.tile([N, C], F32, tag="o_dram")

        # v/kT live through attention; released before proj.  kT's pool opens
        # at phase K so phase V has room to preload wk alongside wv.
        kv_ctx = ctx.enter_context(ExitStack())
        vp = kv_ctx.enter_context(tc.tile_pool(name="vp", bufs=1))
        # v [m,d] as 16 col-blocks of [128, C]
        v = vp.tile([128, MT * C], F32R, tag="v")
        wk_ctx = ctx.enter_context(ExitStack())
        wkp = wk_ctx.enter_context(tc.tile_pool(name="wkp", bufs=1, side="right"))
        wk = wkp.tile([128, CT * C], F32R, tag="wk")

        # ---------------- phase V: v[m, d] = mask * (support @ wv^T) -------
        # wv is dc-major: [p, dc*4096 + ct*512 + dd]
        with (
            tc.tile_pool(name="wvp", bufs=1) as wvp,
            tc.tile_pool(name="stv", bufs=2) as stp,
            tc.tile_pool(name="vps", bufs=3, space="PSUM") as vps,
        ):
            wv = wvp.tile([128, CT * C], F32R, tag="wv")
            st0 = stp.tile([128, CT, MS], F32R, tag="st")
            cw = (CT * C) // WCH
            for i in range(CT):
                nc.sync.dma_start(
                    wv[:, i * cw:(i + 1) * cw],
                    wvT[:, i * cw:(i + 1) * cw].bitcast(F32R),
                )
                nc.sync.dma_start(
                    st0[:, i, :], sT[:, i, 0:MS].bitcast(F32R)
                )
            _dma_w(nc, wv, wvT, CT, WCH)
            for mc in range(M // MS):
                if mc == 0:
                    st = st0
                else:
                    st = stp.tile([128, CT, MS], F32R, tag="st")
                    _dma_act(nc, st, sT, mc * MS, MS)
                # spread next phase's weight prefetch across V's mc loop
                _dma_w(nc, wk, wkT, mc * 4, (mc + 1) * 4)
                for dc in range(C // 512):
                    for j in range(MS // 128):
                        mt = mc * (MS // 128) + j
                        ps = vps.tile([128, 512], F32, tag="vps")
                        for ct in range(CT):
                            nc.tensor.matmul(
                                ps[:],
                                lhsT=st[:, ct, j * 128:(j + 1) * 128],
                                rhs=wv[:, dc * 4096 + ct * 512: dc * 4096 + (ct + 1) * 512],
                                start=(ct == 0),
                                stop=(ct == CT - 1),
                            )
                        nc.vector.tensor_scalar_mul(
                            v[:, mt * C + dc * 512: mt * C + (dc + 1) * 512],
                            ps[:],
                            maskt[:, mt:mt + 1],
                        )

        # ---------------- phase K: kT[d, m] = (support @ wk^T)^T -----------
        # wk is dt-major: [p, dt*1024 + ct*128 + dd]; preloaded during V
        kTp = kv_ctx.enter_context(tc.tile_pool(name="kTp", bufs=1))
        # kT [d,m] as 8 col-blocks of [128, M]
        kT = kTp.tile([128, CT * M], F32R, tag="kT")
        with (
            tc.tile_pool(name="stk", bufs=2) as stp,
            tc.tile_pool(name="kps", bufs=3, space="PSUM") as kps,
        ):
            for mc in range(M // MS):
                st = stp.tile([128, CT, MS], F32R, tag="st")
                _dma_act(nc, st, sT, mc * MS, MS)
                for dt in range(CT):
                    ps = kps.tile([128, MS], F32, tag="kps")
                    for ct in range(CT):
                        nc.tensor.matmul(
                            ps[:],
                            lhsT=wk[:, dt * C + ct * 128: dt * C + (ct + 1) * 128],
                            rhs=st[:, ct, :],
                            start=(ct == 0),
                            stop=(ct == CT - 1),
                        )
                    nc.scalar.copy(
                        kT[:, dt * M + mc * MS: dt * M + (mc + 1) * MS], ps[:]
                    )

        wk_ctx.close()

        # ---------------- attention: per n-chunk qT, then s/p/o ------------
        # wq is dt-major like wk
        with (
            tc.tile_pool(name="wqp", bufs=1) as wqp,
            tc.tile_pool(name="xq", bufs=1) as xqp,
            tc.tile_pool(name="qt", bufs=1) as qtp,
            tc.tile_pool(name="qps", bufs=2, space="PSUM") as qps,
            tc.tile_pool(name="sps", bufs=2, space="PSUM") as sps,
            tc.tile_pool(name="ptps", bufs=2, space="PSUM") as ptps,
            tc.tile_pool(name="ops", bufs=1, space="PSUM") as ops,
            tc.tile_pool(name="psb", bufs=2) as psbp,
            tc.tile_pool(name="ptsb", bufs=2) as ptsbp,
            tc.tile_pool(name="osb", bufs=2) as osbp,
            tc.tile_pool(name="stat", bufs=4) as statp,
        ):
            wq = wqp.tile([128, CT * C], F32R, tag="wq")
            _dma_w(nc, wq, wqT, 0, 2)  # dt0 block: first qT group's weights
            xq0 = xqp.tile([128, CT, NCHUNK], F32R, tag="xq")
            _dma_act(nc, xq0, xT, 0, NCHUNK)
            _dma_w(nc, wq, wqT, 2, WCH)
            for nch in range(N // NCHUNK):
                if nch == 0:
                    xq = xq0
                else:
                    xq = xqp.tile([128, CT, NCHUNK], F32R, tag="xq")
                    _dma_act(nc, xq, xT, nch * NCHUNK, NCHUNK)
                qt = qtp.tile([128, CT * NCHUNK], F32R, tag="qt")
                for dt in range(CT):
                    ps = qps.tile([128, NCHUNK], F32, tag="qps")
                    for ct in range(CT):
                        nc.tensor.matmul(
                            ps[:],
                            lhsT=wq[:, dt * C + ct * 128: dt * C + (ct + 1) * 128],
                            rhs=xq[:, ct, :],
                            start=(ct == 0),
                            stop=(ct == CT - 1),
                        )
                    nc.scalar.copy(
                        qt[:, dt * NCHUNK:(dt + 1) * NCHUNK], ps[:]
                    )
                for nt2 in range(NCHUNK // 128):
                    ntile = nch * (NCHUNK // 128) + nt2
                    partials = statp.tile([128, 4], F32, tag="partials")
                    o_ps = ops.tile([128, C], F32, tag="ops")
                    for g in range(M // MS):
                        s_ps = sps.tile([128, MS], F32, tag="sps")
                        for dt in range(CT):
                            nc.tensor.matmul(
                                s_ps[:],
                                lhsT=qt[:, dt * NCHUNK + nt2 * 128: dt * NCHUNK + (nt2 + 1) * 128],
                                rhs=kT[:, dt * M + g * MS: dt * M + (g + 1) * MS],
                                start=(dt == 0),
                                stop=(dt == CT - 1),
                            )
                        p_sb = psbp.tile([128, MS], F32R, tag="psb")
                        nc.scalar.activation(
                            p_sb[:], s_ps[:], AF.Exp,
                            scale=float(SCALE),
                            accum_out=partials[:, g:g + 1],
                        )
                        pt_ps = ptps.tile([128, MS], F32R, tag="ptps")
                        for j in range(MS // 128):
                            nc.tensor.transpose(
                                pt_ps[:, j * 128:(j + 1) * 128],
                                p_sb[:, j * 128:(j + 1) * 128],
                                ident[:],
                            )
                        pt_sb = ptsbp.tile([128, MS], F32R, tag="ptsb")
                        nc.vector.tensor_copy(pt_sb[:], pt_ps[:])
                        for j in range(MS // 128):
                            mt = g * (MS // 128) + j
                            for dc in range(C // 512):
                                nc.tensor.matmul(
                                    o_ps[:, dc * 512:(dc + 1) * 512],
                                    lhsT=pt_sb[:, j * 128:(j + 1) * 128],
                                    rhs=v[:, mt * C + dc * 512: mt * C + (dc + 1) * 512],
                                    start=(mt == 0),
                                    stop=(mt == MT - 1),
                                )
                    denom = statp.tile([128, 1], F32, tag="denom")
                    nc.vector.reduce_sum(
                        denom[:], partials[:], axis=mybir.AxisListType.X
                    )
                    recip = statp.tile([128, 1], F32, tag="recip")
                    nc.vector.reciprocal(recip[:], denom[:])
                    o_sb = osbp.tile([128, C], F32, tag="osb")
                    nc.vector.tensor_scalar_mul(o_sb[:], o_ps[:], recip[:])
                    nc.sync.dma_start(
                        o_dram[ntile * 128:(ntile + 1) * 128, :], o_sb[:]
                    )

        kv_ctx.close()

        # ---------------- projection with the swapaxes/reshape fold --------
        # pw is dc-major like wv
        with (
            tc.tile_pool(name="pwp", bufs=1) as pwp,
            tc.tile_pool(name="bp", bufs=1) as bp,
            tc.tile_pool(name="obp", bufs=2) as obp,
            tc.tile_pool(name="fps", bufs=2, space="PSUM") as fps,
            tc.tile_pool(name="fsb", bufs=2) as fsbp,
        ):
            pw = pwp.tile([128, CT * C], F32R, tag="pw")
            bias = bp.tile([128, C], F32, tag="bias")
            ob0 = obp.tile([128, CT * C], F32R, tag="ob")
            cw = (CT * C) // WCH
            for i in range(CT):
                nc.sync.dma_start(
                    pw[:, i * cw:(i + 1) * cw],
                    pwT[:, i * cw:(i + 1) * cw].bitcast(F32R),
                )
                # plain 2D slices: a rearranged AP on a DRAM pool tile defeats
                # Tile's RAW dep tracking (read would race the o_dram writes)
                nc.sync.dma_start(
                    ob0[:, i * C:(i + 1) * C],
                    o_dram[i * 128:(i + 1) * 128, :].bitcast(F32R),
                )
            _dma_w(nc, pw, pwT, CT, WCH)
            nc.sync.dma_start(bias[:], biasb[:])
            out_v = out[:].rearrange("(t two) d -> two t d", two=2)
            for i in range(2):
                if i == 0:
                    ob = ob0
                else:
                    ob = obp.tile([128, CT * C], F32R, tag="ob")
                    for ct in range(CT):
                        nc.sync.dma_start(
                            ob[:, ct * C:(ct + 1) * C],
                            o_dram[i * C + ct * 128: i * C + (ct + 1) * 128, :].bitcast(F32R),
                        )
                for dc in range(C // 512):
                    for tt in range(CT):
                        ps = fps.tile([128, 512], F32, tag="fps")
                        for ct in range(CT):
                            nc.tensor.matmul(
                                ps[:],
                                lhsT=ob[:, ct * C + tt * 128: ct * C + (tt + 1) * 128],
                                rhs=pw[:, dc * 4096 + ct * 512: dc * 4096 + (ct + 1) * 512],
                                start=(ct == 0),
                                stop=(ct == CT - 1),
                            )
                        f_sb = fsbp.tile([128, 512], F32, tag="fsb")
                        nc.vector.tensor_add(
                            f_sb[:], ps[:], bias[:, dc * 512:(dc + 1) * 512]
                        )
                        nc.sync.dma_start(
                            out_v[i, tt * 128:(tt + 1) * 128, dc * 512:(dc + 1) * 512],
                            f_sb[:],
                        )


def _prep_w_lhs(w):
    # lhsT weights (wk, wq): dt-major [p, dt*1024 + ct*128 + dd]
    wt = w.T.reshape(CT, 128, CT, 128)          # [ct, p, dt, dd]
    return np.ascontiguousarray(
        wt.transpose(1, 2, 0, 3).reshape(128, CT * C)
    )


def _prep_w_rhs(w):
    # rhs weights (wv, pw): dc-major [p, dc*4096 + ct*512 + dd]
    wt = w.T.reshape(CT, 128, C // 512, 512)    # [ct, p, dc, dd]
    return np.ascontiguousarray(
        wt.transpose(1, 2, 0, 3).reshape(128, CT * C)
    )


def _prep_act(a):
    # a [rows, C] -> a.T [C, rows] grouped as [p, ct, rows]
    n = a.shape[0]
    return np.ascontiguousarray(a.T.reshape(CT, 128, n).transpose(1, 0, 2))


def prep_in_maps(x, support, attn_mask, qkv_w, proj_w, proj_b):
    x = np.asarray(x, dtype=np.float32)
    support = np.asarray(support, dtype=np.float32)
    attn_mask = np.asarray(attn_mask)
    qkv_w = np.asarray(qkv_w, dtype=np.float32)
    proj_w = np.asarray(proj_w, dtype=np.float32)
    proj_b = np.asarray(proj_b, dtype=np.float32)

    wq = _prep_w_lhs(qkv_w[:C])
    wk = _prep_w_lhs(qkv_w[C:2 * C])
    wv = _prep_w_rhs(qkv_w[2 * C:])
    pw = _prep_w_rhs(proj_w)
    maskf = np.ascontiguousarray(
        attn_mask.astype(np.float32).reshape(MT, 128).T
    )
    biasb = np.ascontiguousarray(np.broadcast_to(proj_b, (128, C)))

    in_maps = []
    for b in range(B):
        in_maps.append({
            "xT": _prep_act(x[b]),
            "sT": _prep_act(support[b]),
            "wqT": wq,
            "wkT": wk,
            "wvT": wv,
            "pwT": pw,
            "maskf": maskf,
            "biasb": biasb,
        })
    return in_maps


def kernel(x, support, attn_mask, qkv_w, proj_w, proj_b):
    if "nc" not in _CACHE:
        _CACHE["nc"] = _build_program()
    nc = _CACHE["nc"]

    in_maps = prep_in_maps(x, support, attn_mask, qkv_w, proj_w, proj_b)
    res = run_bass_kernel_spmd(nc, in_maps, core_ids=list(range(B)))
    return np.stack([res.results[b]["out"] for b in range(B)], axis=0)



# revision 6
# speedup vs baseline: 1.3682x; 1.3682x over previous
"""Trainium2 Bass kernel for CrossAttention (B=8, N=M=2048, C=1024), fp32 io.

Sharding: data-parallel — one batch element per NeuronCore (8 cores).

Per-core pipeline (batch b), all matmuls bf16 (1 cyc/row) or fp8-e4m3
DoubleRow 3-term hi/lo (0.75x bf16 cost), fp32 PSUM accumulation:

  kT[d, m] = (support_perm @ wk^T)^T      fp8 DoubleRow, host-prepped operands
  v[m, d]  = mask_perm/32 * (support_perm @ wv^T)   (only mask=1 m-tiles)
  per n-chunk:
    qT[d, n] = (x @ wq^T)^T               fp8 DoubleRow
    sT[m, n] = kT^T qT   (psum, per m-tile; no transposes anywhere)
    pT = exp(SCALE' * sT) -> bf16         (no max-sub: logits <= ~16 in fp32)
    o[n, d] = sum over mask=1 m-tiles of pT^T @ v      (psum)
    den[n]  = ones-matmul column-sum of pT over ALL m-tiles (free-dim-1
              matmuls are ~free); o_sb = o * 1/den  -> bf16
  per n-half: proj with the swapaxes/reshape fold (contraction over o rows),
              bias add, DMA out.

support rows are permuted on host so mask=1 rows come first: the post-softmax
column mask makes masked-out columns contribute only to the softmax
denominator, so p@v and the v build skip them entirely (exact, not approx).
Weights are scaled x32 on host so fp8 hi/lo residuals stay in e4m3 normal
range; the 1/32 factors fold into the exp scale and the mask multiplier.
"""

import sys

sys.path.insert(0, "/opt/trn_rl_repo")

import numpy as np

import concourse.bass as bass
import concourse.tile as tile
from concourse import bacc, mybir
from concourse.bass_utils import run_bass_kernel_spmd

F32 = mybir.dt.float32
BF16 = mybir.dt.bfloat16
F8 = mybir.dt.float8e4
AF = mybir.ActivationFunctionType
PM = mybir.MatmulPerfMode.DoubleRow
F8NP = mybir.dt.np(F8)
BFNP = mybir.dt.np(BF16)

B, N, M, C = 8, 2048, 2048, 1024
CT = C // 128            # 8 contraction tiles
NF = 512                 # n-cols per attention chunk
NCH = N // NF            # 4 chunks
SCALE = (C // 8) ** -0.5
WS = 32.0                # host weight scale (keeps fp8 lo-split in normal range)
EXPSCALE = float(SCALE / (WS * WS))

_CACHE = {}


def _dr3(nc, ps, ah, al, bh, bl, first, last):
    """3-term fp8 DoubleRow accumulation block: (ah+al)^T(bh+bl) minus lo*lo.
    ah/al stationary slices [128, 2, <=128]; bh/bl moving [128, 2, <=256]."""
    terms = ((ah, bh), (ah, bl), (al, bh))
    for i, (a, b) in enumerate(terms):
        nc.tensor.matmul(
            ps, lhsT=a, rhs=b,
            start=(first and i == 0),
            stop=(last and i == 2),
            perf_mode=PM,
        )


def _build_program(mt_in):
    nc = bacc.Bacc(
        "TRN2",
        target_bir_lowering=False,
        debug=False,
        enable_asserts=False,
        num_devices=8,
    )

    x8h = nc.dram_tensor("x8h", [128, CT, N], F8, kind="ExternalInput")
    x8l = nc.dram_tensor("x8l", [128, CT, N], F8, kind="ExternalInput")
    s8h = nc.dram_tensor("s8h", [128, CT, M], F8, kind="ExternalInput")
    s8l = nc.dram_tensor("s8l", [128, CT, M], F8, kind="ExternalInput")
    wq8h = nc.dram_tensor("wq8h", [128, CT, C], F8, kind="ExternalInput")
    wq8l = nc.dram_tensor("wq8l", [128, CT, C], F8, kind="ExternalInput")
    wk8h = nc.dram_tensor("wk8h", [128, CT, C], F8, kind="ExternalInput")
    wk8l = nc.dram_tensor("wk8l", [128, CT, C], F8, kind="ExternalInput")
    wv8h = nc.dram_tensor("wv8h", [128, CT, C], F8, kind="ExternalInput")
    wv8l = nc.dram_tensor("wv8l", [128, CT, C], F8, kind="ExternalInput")
    pwb = nc.dram_tensor("pwb", [128, CT, C], BF16, kind="ExternalInput")
    maskf = nc.dram_tensor("maskf", [128, max(mt_in, 1)], F32,
                           kind="ExternalInput")
    biasb = nc.dram_tensor("biasb", [128, C], F32, kind="ExternalInput")
    out = nc.dram_tensor("out", [N, C], F32, kind="ExternalOutput")

    with tile.TileContext(nc, pool_alloc_mode="queue") as tc:
        _trace_kernel(tc, mt_in, x8h, x8l, s8h, s8l, wq8h, wq8l, wk8h, wk8l,
                      wv8h, wv8l, pwb, maskf, biasb, out)
    nc.compile()
    return nc


def _trace_kernel(tc, mt_in, x8h, x8l, s8h, s8l, wq8h, wq8l, wk8h, wk8l,
                  wv8h, wv8l, pwb, maskf, biasb, out):
    nc = tc.nc
    from contextlib import ExitStack

    MT = M // 128

    with ExitStack() as ctx:
        persist = ctx.enter_context(tc.tile_pool(name="persist", bufs=1))
        maskt = persist.tile([128, max(mt_in, 1)], F32, tag="maskt")
        nc.sync.dma_start(maskt[:], maskf[:])
        ones = persist.tile([128, 1], BF16, tag="ones")
        nc.vector.memset(ones[:], 1.0)
        bias = persist.tile([128, C], F32, tag="bias")
        nc.sync.dma_start(bias[:], biasb[:])

        # persistent activation-derived tensors
        kT = persist.tile([128, CT, M], BF16, tag="kT")
        v = persist.tile([128, max(mt_in, 1), C], BF16, tag="v")
        pwt = persist.tile([128, CT, C], BF16, tag="pwt")

        # ------------- build phase: kT and v (fp8 DoubleRow 3-term) -------
        with (
            tc.tile_pool(name="w8", bufs=1) as w8p,
            tc.tile_pool(name="sp8", bufs=1) as sp8,
            tc.tile_pool(name="bld", bufs=3, space="PSUM") as bld,
        ):
            wkh = w8p.tile([128, CT, C], F8, tag="wkh")
            wkl = w8p.tile([128, CT, C], F8, tag="wkl")
            wvh = w8p.tile([128, CT, C], F8, tag="wvh")
            wvl = w8p.tile([128, CT, C], F8, tag="wvl")
            sph = sp8.tile([128, CT, M], F8, tag="sph")
            spl = sp8.tile([128, CT, M], F8, tag="spl")
            # first chunk of support + wk before anything else
            for ct in range(CT):
                nc.sync.dma_start(sph[:, ct, 0:256], s8h[:, ct, 0:256])
                nc.sync.dma_start(spl[:, ct, 0:256], s8l[:, ct, 0:256])
            nc.sync.dma_start(wkh[:], wk8h[:])
            nc.sync.dma_start(wkl[:], wk8l[:])
            nc.sync.dma_start(wvh[:], wv8h[:])
            nc.sync.dma_start(wvl[:], wv8l[:])
            for mc in range(1, M // 256):
                for ct in range(CT):
                    nc.sync.dma_start(
                        sph[:, ct, mc * 256:(mc + 1) * 256],
                        s8h[:, ct, mc * 256:(mc + 1) * 256])
                    nc.sync.dma_start(
                        spl[:, ct, mc * 256:(mc + 1) * 256],
                        s8l[:, ct, mc * 256:(mc + 1) * 256])

            for mc in range(M // 256):
                sl = slice(mc * 256, (mc + 1) * 256)
                # kT[d, m] for this m-chunk: per d-tile
                for dt in range(CT):
                    ps = bld.tile([128, 512], F32, tag="bld")
                    for pr in range(CT // 2):
                        _dr3(nc, ps[:, 0:256],
                             wkh[:, 2 * pr:2 * pr + 2, dt * 128:(dt + 1) * 128],
                             wkl[:, 2 * pr:2 * pr + 2, dt * 128:(dt + 1) * 128],
                             sph[:, 2 * pr:2 * pr + 2, sl],
                             spl[:, 2 * pr:2 * pr + 2, sl],
                             pr == 0, pr == CT // 2 - 1)
                    nc.scalar.copy(kT[:, dt, sl], ps[:, 0:256])
                # v[m, d] for this chunk's masked-in m-tiles
                for j in range(2):
                    mt = mc * 2 + j
                    if mt >= mt_in:
                        continue
                    for dc in range(C // 256):
                        ps = bld.tile([128, 512], F32, tag="bld")
                        dsl = slice(dc * 256, (dc + 1) * 256)
                        for pr in range(CT // 2):
                            _dr3(nc, ps[:, 0:256],
                                 sph[:, 2 * pr:2 * pr + 2,
                                     mt * 128:(mt + 1) * 128],
                                 spl[:, 2 * pr:2 * pr + 2,
                                     mt * 128:(mt + 1) * 128],
                                 wvh[:, 2 * pr:2 * pr + 2, dsl],
                                 wvl[:, 2 * pr:2 * pr + 2, dsl],
                                 pr == 0, pr == CT // 2 - 1)
                        nc.vector.tensor_scalar_mul(
                            v[:, mt, dsl], ps[:, 0:256], maskt[:, mt:mt + 1])

        # ------------- attention + interleaved projection ------------------
        with (
            tc.tile_pool(name="wq8", bufs=1) as wq8p,
            tc.tile_pool(name="x8", bufs=2) as x8p,
            tc.tile_pool(name="qt", bufs=2) as qtp,
            tc.tile_pool(name="pt", bufs=18) as ptp,
            tc.tile_pool(name="ob", bufs=10) as obp,
            tc.tile_pool(name="fo", bufs=3) as fop,
            tc.tile_pool(name="st", bufs=4) as stp,
            tc.tile_pool(name="qps", bufs=2, space="PSUM") as qps,
            tc.tile_pool(name="sps", bufs=2, space="PSUM") as sps,
            tc.tile_pool(name="ops", bufs=2, space="PSUM") as ops,
            tc.tile_pool(name="djs", bufs=2, space="PSUM") as djs,
        ):
            wqh = wq8p.tile([128, CT, C], F8, tag="wqh")
            wql = wq8p.tile([128, CT, C], F8, tag="wql")
            nc.sync.dma_start(wqh[:], wq8h[:])
            nc.sync.dma_start(wql[:], wq8l[:])
            nc.sync.dma_start(pwt[:], pwb[:])

            x8 = [None, None]

            def load_x(c):
                xh = x8p.tile([128, CT, NF], F8, tag="xh")
                xl = x8p.tile([128, CT, NF], F8, tag="xl")
                nsl = slice(c * NF, (c + 1) * NF)
                for ct in range(CT):
                    nc.sync.dma_start(xh[:, ct, :], x8h[:, ct, nsl])
                    nc.sync.dma_start(xl[:, ct, :], x8l[:, ct, nsl])
                return xh, xl

            x8[0] = load_x(0)
            x8[1] = load_x(1)

            out_v = out[:].rearrange("(t two) d -> two t d", two=2)
            o_half = [[None] * 8, [None] * 8]

            for c in range(NCH):
                xh, xl = x8[c % 2]
                # qT for this chunk (fp8 DoubleRow 3-term)
                qt = qtp.tile([128, CT, NF], BF16, tag="qt")
                for dt in range(CT):
                    for nh in range(NF // 256):
                        ps = qps.tile([128, 512], F32, tag="qps")
                        for pr in range(CT // 2):
                            _dr3(nc, ps[:, 0:256],
                                 wqh[:, 2 * pr:2 * pr + 2,
                                     dt * 128:(dt + 1) * 128],
                                 wql[:, 2 * pr:2 * pr + 2,
                                     dt * 128:(dt + 1) * 128],
                                 xh[:, 2 * pr:2 * pr + 2,
                                    nh * 256:(nh + 1) * 256],
                                 xl[:, 2 * pr:2 * pr + 2,
                                    nh * 256:(nh + 1) * 256],
                                 pr == 0, pr == CT // 2 - 1)
                        nc.scalar.copy(
                            qt[:, dt, nh * 256:(nh + 1) * 256], ps[:, 0:256])
                if c + 2 < NCH:
                    x8[c % 2] = load_x(c + 2)

                # sT per m-tile, exp -> pT (bf16)
                pts = []
                for mt in range(MT):
                    ps = sps.tile([128, NF], F32, tag="sps")
                    for dt in range(CT):
                        nc.tensor.matmul(
                            ps[:],
                            lhsT=kT[:, dt, mt * 128:(mt + 1) * 128],
                            rhs=qt[:, dt, :],
                            start=(dt == 0),
                            stop=(dt == CT - 1),
                        )
                    pt = ptp.tile([128, NF], BF16, tag="pt")
                    nc.scalar.activation(pt[:], ps[:], AF.Exp, scale=EXPSCALE)
                    pts.append(pt)

                # p@v + denominator + normalize, per n-tile of 128
                for nt in range(NF // 128):
                    ntile = c * (NF // 128) + nt
                    nsl = slice(nt * 128, (nt + 1) * 128)
                    o_ps = []
                    for dh in range(2):
                        ps = ops.tile([128, 512], F32, tag="ops")
                        for mt in range(mt_in):
                            nc.tensor.matmul(
                                ps[:],
                                lhsT=pts[mt][:, nsl],
                                rhs=v[:, mt, dh * 512:(dh + 1) * 512],
                                start=(mt == 0),
                                stop=(mt == mt_in - 1),
                            )
                        o_ps.append(ps)
                    dn = djs.tile([128, 512], F32, tag="djs")
                    for mt in range(MT):
                        nc.tensor.matmul(
                            dn[:, 0:1],
                            lhsT=pts[mt][:, nsl],
                            rhs=ones[:],
                            start=(mt == 0),
                            stop=(mt == MT - 1),
                        )
                    recip = stp.tile([128, 1], F32, tag="recip")
                    nc.vector.reciprocal(recip[:], dn[:, 0:1])
                    ob = obp.tile([128, C], BF16, tag="ob")
                    for dh in range(2):
                        nc.vector.tensor_scalar_mul(
                            ob[:, dh * 512:(dh + 1) * 512], o_ps[dh][:],
                            recip[:])
                    o_half[ntile // 8][ntile % 8] = ob

                # after each half: projection with the swapaxes fold
                if c % 2 == 1:
                    h = c // 2
                    otiles = o_half[h]
                    for tt in range(CT):
                        for dc in range(2):
                            ps = djs.tile([128, 512], F32, tag="djs")
                            for ct in range(CT):
                                nc.tensor.matmul(
                                    ps[:],
                                    lhsT=otiles[ct][:, tt * 128:(tt + 1) * 128],
                                    rhs=pwt[:, ct, dc * 512:(dc + 1) * 512],
                                    start=(ct == 0),
                                    stop=(ct == CT - 1),
                                )
                            f_sb = fop.tile([128, 512], F32, tag="fo")
                            nc.vector.tensor_add(
                                f_sb[:], ps[:], bias[:, dc * 512:(dc + 1) * 512])
                            nc.sync.dma_start(
                                out_v[h, tt * 128:(tt + 1) * 128,
                                      dc * 512:(dc + 1) * 512],
                                f_sb[:],
                            )


def _prep_layout(a):
    # a [rows(c), cols] -> [128, CT, cols] with c = ct*128 + p
    cols = a.shape[1]
    return np.ascontiguousarray(
        a.reshape(CT, 128, cols).transpose(1, 0, 2))


def _hl(a):
    hi = a.astype(F8NP)
    lo = (a - hi.astype(np.float32)).astype(F8NP)
    return np.ascontiguousarray(hi), np.ascontiguousarray(lo)


def prep_in_maps(x, support, attn_mask, qkv_w, proj_w, proj_b):
    x = np.asarray(x, dtype=np.float32)
    support = np.asarray(support, dtype=np.float32)
    attn_mask = np.asarray(attn_mask)
    qkv_w = np.asarray(qkv_w, dtype=np.float32)
    proj_w = np.asarray(proj_w, dtype=np.float32)
    proj_b = np.asarray(proj_b, dtype=np.float32)

    mask = (attn_mask != 0)
    perm = np.argsort(~mask, kind="stable")
    m1 = int(mask.sum())
    mt_in = max((m1 + 127) // 128, 1)
    mask_perm = mask[perm].astype(np.float32)

    wq = qkv_w[:C] * WS
    wk = qkv_w[C:2 * C] * WS
    wv = qkv_w[2 * C:] * WS
    wq8h, wq8l = _hl(_prep_layout(wq.T))
    wk8h, wk8l = _hl(_prep_layout(wk.T))
    wv8h, wv8l = _hl(_prep_layout(wv.T))
    pwb = np.ascontiguousarray(_prep_layout(proj_w.T).astype(BFNP))
    maskf = np.ascontiguousarray(
        (mask_perm[:mt_in * 128] / WS).reshape(mt_in, 128).T)
    biasb = np.ascontiguousarray(
        np.broadcast_to(proj_b, (128, C)).astype(np.float32))

    in_maps = []
    for b in range(B):
        x8h, x8l = _hl(_prep_layout(x[b].T))
        s8h, s8l = _hl(_prep_layout(support[b][perm].T))
        in_maps.append({
            "x8h": x8h, "x8l": x8l, "s8h": s8h, "s8l": s8l,
            "wq8h": wq8h, "wq8l": wq8l, "wk8h": wk8h, "wk8l": wk8l,
            "wv8h": wv8h, "wv8l": wv8l, "pwb": pwb,
            "maskf": maskf, "biasb": biasb,
        })
    return in_maps, mt_in


def kernel(x, support, attn_mask, qkv_w, proj_w, proj_b):
    in_maps, mt_in = prep_in_maps(x, support, attn_mask, qkv_w, proj_w,
                                  proj_b)
    if mt_in not in _CACHE:
        _CACHE[mt_in] = _build_program(mt_in)
    nc = _CACHE[mt_in]
    _CACHE["nc"] = nc

    res = run_bass_kernel_spmd(nc, in_maps, core_ids=list(range(B)))
    return np.stack([res.results[b]["out"] for b in range(B)], axis=0)
